# revision 21
# baseline (speedup 1.0000x reference)
"""MinibatchDiscrimination TRN2 Bass kernel.

Math (per sample n, kernels K=32, dim D=16, features F=64):
  M = x @ T                      (N, K*D)
  A[n,k,d] = sum_j |M[n,j,d] - M[n,k,d]|
  feats[n,k] = sum_d exp(-A[n,k,d])
  out = concat([x, feats], -1)   (N, F+K)

Strategy: data-parallel over 8 cores (512 samples each). On each core the
pairwise reduction is decomposed as matmuls around a single elementwise pass:
  Dif[p, n]  = M[a_p, d, n] - M[b_p, d, n]   (PE: +/-1 matrix, pairs a<b)
  P[p, n]    = |Dif[p, n]|                   (ACT/DVE, PSUM->SBUF)
  A[k, d, n] = sum_p E2[p, k] * P[p, n]      (PE: 0/1 matrix)
  feats[k,n] = sum_d exp(-A)                 (ACT exp, PE selection matmul)
Exploiting |a-b| symmetry halves the elementwise work (496 pairs vs 1024).
"""

import json
import os
from contextlib import ExitStack

import numpy as np
import ml_dtypes

import concourse.bass as bass
import concourse.tile as tile
from concourse import mybir
from concourse.vector_clock import ScopedClock
from concourse.bass_utils import run_bass_kernel_spmd
from concourse.masks import make_identity

K, D, F = 32, 16, 64
KD = K * D                      # 512
NS = 512                        # samples per core
NCORES = 8
NPAIRS = K * (K - 1) // 2       # 496
NCHUNK = 4                      # pair chunks
CHROWS = 124                    # pairs per chunk (<=128)

F32 = mybir.dt.float32
BF16 = mybir.dt.bfloat16
NPBF16 = ml_dtypes.bfloat16

# Fraction of |.| ops on DVE vs ACT (DVE uses relu(x)+relu(-x) split, see below)
_ABS_ON_DVE = ()  # v1: all abs on ACT


def _split_multiwait_json(bj: bytes) -> bytes:
    """This container's walrus rejects instructions carrying >1 sync wait.
    Hoist extra waits into single-wait EventSemaphore carriers placed just
    before the instruction (same engine => same sequencer stream position).
    Only monotonic sem-ge waits are hoisted; order-sensitive modes (the
    barrier's sem-eq-0) stay attached."""
    d = json.loads(bj)
    ctr = 0
    for f in d["functions"]:
        for b in f["blocks"]:
            new = []
            for inst in b["instructions"]:
                si = inst.get("sync_info")
                waits = (si or {}).get("on_wait") or []
                if len(waits) > 1:
                    eng = inst.get("engine")
                    assert eng, f"no engine on multiwait inst {inst.get('name')}"
                    hoist = [w for w in waits if w.get("wait_mode") == "sem-ge-imm"]
                    keep = [w for w in waits if w.get("wait_mode") != "sem-ge-imm"]
                    # keep at most one wait attached to the instruction itself
                    if not keep and hoist:
                        keep = [hoist.pop()]
                    assert len(keep) <= 1, f"unsplittable waits on {inst.get('name')}"
                    for w in hoist:
                        ctr += 1
                        new.append(
                            {
                                "debug": inst.get("debug", 0),
                                "engine": eng,
                                "ins": [],
                                "outs": [],
                                "name": f"hoistw-{ctr}",
                                "opcode": "EventSemaphore",
                                "sync_info": {"on_update": [], "on_wait": [w]},
                            }
                        )
                    si["on_wait"] = keep
                new.append(inst)
            b["instructions"] = new
    return json.dumps(d).encode()


def _patch_to_json():
    if getattr(bass.Bass, "_multiwait_patched", False):
        return
    orig = bass.Bass.to_json_bytes

    def to_json_bytes(self):
        return _split_multiwait_json(orig(self))

    bass.Bass.to_json_bytes = to_json_bytes
    bass.Bass._multiwait_patched = True


def _host_constants():
    """Constant matrices shipped to every core."""
    pairs = [(a, b) for a in range(K) for b in range(a + 1, K)]
    # W[j', c*128 + r]: MM1 lhsT. Column (c,r) encodes pair p=(a,b):
    #   out[r, n] = M_T2[32d+a, n] - M_T2[32d+b, n]
    W = np.zeros((K, NCHUNK * 128), np.float32)
    # E2[r, c*32 + k]: MM2 lhsT. Pair (a,b) contributes |Dif| to A[a] and A[b].
    E2 = np.zeros((128, NCHUNK * K), np.float32)
    for p, (a, b) in enumerate(pairs):
        c, r = divmod(p, CHROWS)
        W[a, c * 128 + r] = 1.0
        W[b, c * 128 + r] = -1.0
        E2[r, c * K + a] = 1.0
        E2[r, c * K + b] = 1.0
    # Replicate W at the 4 row strips so MM1 for d can use row group d%4.
    W_rep = np.tile(W, (4, 1))  # (128, 512)
    # Sel[(gi, k'), k] = (k'==k): MM3 lhsT, sums exp over the 4 d's per bank.
    Sel = np.zeros((128, K), np.float32)
    for gi in range(4):
        for k in range(K):
            Sel[32 * gi + k, k] = 1.0
    return W_rep, E2, Sel


def _build_nc(mm_dt, np_mm_dt):
    """Build the Bass module (same NEFF for all 8 cores)."""
    _patch_to_json()
    nc = bass.Bass("TRN2", enable_partition_id=False)
    x_in = nc.dram_tensor("x", (NS, F), F32, kind="ExternalInput")
    # c64: [xT | Tp] packed; c128: [W | E2 | Sel] packed (1 DMA each)
    c64_in = nc.dram_tensor("c64", (F, NS + KD), mm_dt, kind="ExternalInput")
    c128_in = nc.dram_tensor(
        "c128", (128, NCHUNK * 128 + NCHUNK * K + K), mm_dt, kind="ExternalInput"
    )
    out = nc.dram_tensor("out", (NS, F + K), F32, kind="ExternalOutput")

    with tile.TileContext(nc) as tc, ExitStack() as ctx:
        consts = ctx.enter_context(tc.tile_pool(name="consts", bufs=1))
        mt2_pool = ctx.enter_context(tc.tile_pool(name="mt2", bufs=2))
        pabs_pool = ctx.enter_context(tc.tile_pool(name="pabs", bufs=4))
        exp_pool = ctx.enter_context(tc.tile_pool(name="exps", bufs=2))
        misc_pool = ctx.enter_context(tc.tile_pool(name="misc", bufs=2))
        mm1_ps = ctx.enter_context(tc.tile_pool(name="mm1ps", bufs=3, space="PSUM"))
        a_ps = ctx.enter_context(tc.tile_pool(name="aps", bufs=1, space="PSUM"))
        f_ps = ctx.enter_context(tc.tile_pool(name="fps", bufs=1, space="PSUM"))

        # ---- constants / inputs to SBUF (matmul operands first) ----
        c64_sb = consts.tile([F, NS + KD], mm_dt)
        nc.sync.dma_start(out=c64_sb[:], in_=c64_in[:, :])
        c128_sb = consts.tile([128, NCHUNK * 128 + NCHUNK * K + K], mm_dt)
        nc.sync.dma_start(out=c128_sb[:], in_=c128_in[:, :])
        xT_sb = c64_sb[:, 0:NS]
        tp_sb = c64_sb[:, NS : NS + KD]
        w_sb = c128_sb[:, 0 : NCHUNK * 128]
        e2_sb = c128_sb[:, NCHUNK * 128 : NCHUNK * 128 + NCHUNK * K]
        sel_sb = c128_sb[:, NCHUNK * 128 + NCHUNK * K :]
        ident = consts.tile([128, 128], F32)

        # x passthrough: HBM -> HBM, fully off the critical path
        nc.sync.dma_start(out=out[:, 0:F], in_=x_in[:, :])

        # ---- M_T2[(d*32+j), n] = sum_f Tp[f, d*32+j] * xT[f, n] ----
        # Two MMs per double-wide PSUM slot; one wide copy-cast drains both.
        mt2_sb = []
        for h in range(2):
            ps = mm1_ps.tile([128, 2 * NS], F32, tag="mm1")
            for s in range(2):
                q = 2 * h + s
                nc.tensor.matmul(
                    ps[:, s * NS : (s + 1) * NS],
                    lhsT=tp_sb[:, q * 128 : (q + 1) * 128], rhs=xT_sb[:],
                    start=True, stop=True,
                )
            m = mt2_pool.tile([128, 2 * NS], mm_dt, tag="mt2")
            if h == 0:
                nc.scalar.copy(out=m[:], in_=ps[:])
            else:
                nc.vector.tensor_copy(out=m[:], in_=ps[:])
            mt2_sb.append(m)

        def mt2_slice(q, r):
            return mt2_sb[q // 2][
                32 * r : 32 * r + 32, (q % 2) * NS : (q % 2) * NS + NS
            ]

        # ---- main loop: per d-group q (d = 4q+r), per chunk c ----
        # Software pipeline per chunk: MM1 quad (4-way row-packed, two
        # double-wide PSUM slots) -> two double-wide |.| ops (ACT + DVE in
        # parallel) -> MM2 quad (4-way col-packed) one chunk behind.
        feats_ps = f_ps.tile([K, NS], F32, tag="feats")
        pend = []  # (q, c, pabs tile) awaiting MM2 quad
        a_banks = {}
        mm2_done = {q: 0 for q in range(4)}

        def emit_mm2_quad():
            qq, cc, pp = pend.pop(0)
            for r in range(4):
                nc.tensor.matmul(
                    a_banks[qq][32 * r : 32 * r + 32, :],
                    lhsT=e2_sb[:, cc * K : (cc + 1) * K],
                    rhs=pp[:, r * NS : (r + 1) * NS],
                    start=(cc == 0), stop=(cc == NCHUNK - 1),
                    tile_position=(0, 32 * r),
                )
            mm2_done[qq] += 1
            if mm2_done[qq] == NCHUNK:
                # A(q) complete: exp(-A) then accumulate over d into feats
                ex = exp_pool.tile([128, NS], mm_dt, tag="exps")
                nc.scalar.activation(
                    out=ex[:], in_=a_banks[qq][:],
                    func=mybir.ActivationFunctionType.Exp, scale=-1.0,
                )
                nc.tensor.matmul(
                    feats_ps[:], lhsT=sel_sb[:], rhs=ex[:],
                    start=(qq == 0), stop=(qq == 3),
                )

        for q in range(4):
            a_banks[q] = a_ps.tile([128, NS], F32, tag="abank", name=f"abank_{q}")
            for c in range(NCHUNK):
                p1 = mm1_ps.tile([128, 2 * NS], F32, tag="mm1")
                p2 = mm1_ps.tile([128, 2 * NS], F32, tag="mm1")
                for r in range(4):
                    slot = p1 if r < 2 else p2
                    nc.tensor.matmul(
                        slot[:, (r % 2) * NS : (r % 2 + 1) * NS],
                        lhsT=w_sb[32 * r : 32 * r + 32, c * 128 : (c + 1) * 128],
                        rhs=mt2_slice(q, r),
                        start=True, stop=True,
                        tile_position=(32 * r, 0),
                    )
                pa = pabs_pool.tile([128, 4 * NS], mm_dt, tag="pabs")
                # ACT and DVE each drain half of BOTH slots, so a slot's
                # recycle latency is one half-op, not a full double-wide op.
                for si, slot in enumerate((p1, p2)):
                    nc.scalar.activation(
                        out=pa[:, 2 * si * NS : (2 * si + 1) * NS],
                        in_=slot[:, 0:NS],
                        func=mybir.ActivationFunctionType.Abs,
                    )
                    with nc.allow_low_precision(reason="abs via 1-elem reduce"):
                        nc.vector.tensor_reduce(
                            out=pa[:, (2 * si + 1) * NS : (2 * si + 2) * NS],
                            in_=slot[:, NS : 2 * NS].rearrange(
                                "p (n o) -> p n o", o=1
                            ),
                            axis=mybir.AxisListType.X,
                            op=mybir.AluOpType.add,
                            apply_absolute_value=True,
                        )
                pend.append((q, c, pa))
                if len(pend) > 1:
                    emit_mm2_quad()
        while pend:
            emit_mm2_quad()

        # ---- feats (K, NS) -> out[:, F:F+K] ----
        make_identity(nc, ident[:])
        feats_sb = misc_pool.tile([K, NS], F32, tag="feats_sb")
        nc.vector.tensor_copy(out=feats_sb[:], in_=feats_ps[:])
        fstage = misc_pool.tile([128, 4, K], F32, tag="fstage")
        for t in range(4):
            tp = mm1_ps.tile([128, 2 * NS], F32, tag="mm1")
            nc.tensor.transpose(
                tp[:, :K], feats_sb[:, t * 128 : (t + 1) * 128], ident[:K, :K]
            )
            if t % 2 == 0:
                nc.vector.tensor_copy(out=fstage[:, t, :], in_=tp[:, :K])
            else:
                nc.scalar.copy(out=fstage[:, t, :], in_=tp[:, :K])
        nc.sync.dma_start(
            out=out[:, :].rearrange("(t p) f -> p t f", p=128)[:, :, F : F + K],
            in_=fstage[:],
        )
    return nc


_CACHED = {}


def _get_nc(use_bf16):
    key = ("bf16" if use_bf16 else "f32",)
    if key not in _CACHED:
        mm_dt = BF16 if use_bf16 else F32
        np_dt = NPBF16 if use_bf16 else np.float32
        _CACHED[key] = (_build_nc(mm_dt, np_dt), np_dt)
    return _CACHED[key]


def kernel(x, T, num_kernels, kernel_dim):
    assert int(num_kernels) == K and int(kernel_dim) == D
    x = np.asarray(x, dtype=np.float32)
    T = np.asarray(T, dtype=np.float32)
    B, S, f = x.shape
    assert (B, S, f) == (8, 512, 64) and T.shape == (F, KD)

    use_bf16 = os.environ.get("MBD_MM_DTYPE", "f32") == "bf16"
    nc, np_dt = _get_nc(use_bf16)

    # T_perm[f, d*32 + k] = T[f, k*16 + d]
    T_perm = T.reshape(F, K, D).transpose(0, 2, 1).reshape(F, KD)
    W_rep, E2, Sel = _host_constants()
    c128 = np.ascontiguousarray(
        np.concatenate([W_rep, E2, Sel], axis=1).astype(np_dt)
    )

    in_maps = []
    for c in range(NCORES):
        xc = np.ascontiguousarray(x[c])
        c64 = np.ascontiguousarray(
            np.concatenate([xc.T, T_perm], axis=1).astype(np_dt)
        )
        in_maps.append({"x": xc, "c64": c64, "c128": c128})

    trace = os.environ.get("MBD_TRACE", "0") == "1"
    res = run_bass_kernel_spmd(
        nc, in_maps, core_ids=list(range(NCORES)), trace=trace
    )
    kernel.last_results = res
    return np.stack([res.results[c]["out"] for c in range(NCORES)], axis=0)


# revision 23
# speedup vs baseline: 1.0986x; 1.0986x over previous
"""MinibatchDiscrimination TRN2 Bass kernel.

Math (per sample n, kernels K=32, dim D=16, features F=64):
  M = x @ T                      (N, K*D)
  A[n,k,d] = sum_j |M[n,j,d] - M[n,k,d]|
  feats[n,k] = sum_d exp(-A[n,k,d])
  out = concat([x, feats], -1)   (N, F+K)

Strategy: data-parallel over 8 cores (512 samples each). On each core the
pairwise reduction is decomposed as matmuls around a single elementwise pass:
  Dif[p, n]  = M[a_p, d, n] - M[b_p, d, n]   (PE: +/-1 matrix, pairs a<b)
  P[p, n]    = |Dif[p, n]|                   (ACT/DVE, PSUM->SBUF)
  A[k, d, n] = sum_p E2[p, k] * P[p, n]      (PE: 0/1 matrix)
  feats[k,n] = sum_d exp(-A)                 (ACT exp, PE selection matmul)
Exploiting |a-b| symmetry halves the elementwise work (496 pairs vs 1024).
"""

import json
import os
from contextlib import ExitStack

import numpy as np
import ml_dtypes

import concourse.bass as bass
import concourse.tile as tile
from concourse import mybir
from concourse.vector_clock import ScopedClock
from concourse.bass_utils import run_bass_kernel_spmd
from concourse.masks import make_identity

K, D, F = 32, 16, 64
KD = K * D                      # 512
NS = 512                        # samples per core
NCORES = 8
NPAIRS = K * (K - 1) // 2       # 496
NCHUNK = 4                      # pair chunks
CHROWS = 124                    # pairs per chunk (<=128)

F32 = mybir.dt.float32
BF16 = mybir.dt.bfloat16
NPBF16 = ml_dtypes.bfloat16

# Fraction of |.| ops on DVE vs ACT (DVE uses relu(x)+relu(-x) split, see below)
_ABS_ON_DVE = ()  # v1: all abs on ACT


def _split_multiwait_json(bj: bytes) -> bytes:
    """This container's walrus rejects instructions carrying >1 sync wait.
    Hoist extra waits into single-wait EventSemaphore carriers placed just
    before the instruction (same engine => same sequencer stream position).
    Only monotonic sem-ge waits are hoisted; order-sensitive modes (the
    barrier's sem-eq-0) stay attached."""
    d = json.loads(bj)
    ctr = 0
    for f in d["functions"]:
        for b in f["blocks"]:
            new = []
            for inst in b["instructions"]:
                si = inst.get("sync_info")
                waits = (si or {}).get("on_wait") or []
                if len(waits) > 1:
                    eng = inst.get("engine")
                    assert eng, f"no engine on multiwait inst {inst.get('name')}"
                    hoist = [w for w in waits if w.get("wait_mode") == "sem-ge-imm"]
                    keep = [w for w in waits if w.get("wait_mode") != "sem-ge-imm"]
                    # keep at most one wait attached to the instruction itself
                    if not keep and hoist:
                        keep = [hoist.pop()]
                    assert len(keep) <= 1, f"unsplittable waits on {inst.get('name')}"
                    for w in hoist:
                        ctr += 1
                        new.append(
                            {
                                "debug": inst.get("debug", 0),
                                "engine": eng,
                                "ins": [],
                                "outs": [],
                                "name": f"hoistw-{ctr}",
                                "opcode": "EventSemaphore",
                                "sync_info": {"on_update": [], "on_wait": [w]},
                            }
                        )
                    si["on_wait"] = keep
                new.append(inst)
            b["instructions"] = new
    return json.dumps(d).encode()


def _patch_to_json():
    if getattr(bass.Bass, "_multiwait_patched", False):
        return
    orig = bass.Bass.to_json_bytes

    def to_json_bytes(self):
        return _split_multiwait_json(orig(self))

    bass.Bass.to_json_bytes = to_json_bytes
    bass.Bass._multiwait_patched = True


def _host_constants():
    """Constant matrices shipped to every core."""
    pairs = [(a, b) for a in range(K) for b in range(a + 1, K)]
    # W[j', c*128 + r]: MM1 lhsT. Column (c,r) encodes pair p=(a,b):
    #   out[r, n] = M_T2[32d+a, n] - M_T2[32d+b, n]
    W = np.zeros((K, NCHUNK * 128), np.float32)
    # E2[r, c*32 + k]: MM2 lhsT. Pair (a,b) contributes |Dif| to A[a] and A[b].
    E2 = np.zeros((128, NCHUNK * K), np.float32)
    for p, (a, b) in enumerate(pairs):
        c, r = divmod(p, CHROWS)
        W[a, c * 128 + r] = 1.0
        W[b, c * 128 + r] = -1.0
        E2[r, c * K + a] = 1.0
        E2[r, c * K + b] = 1.0
    # Replicate W at the 4 row strips so MM1 for d can use row group d%4.
    W_rep = np.tile(W, (4, 1))  # (128, 512)
    # Sel[(gi, k'), k] = (k'==k): MM3 lhsT, sums exp over the 4 d's per bank.
    Sel = np.zeros((128, K), np.float32)
    for gi in range(4):
        for k in range(K):
            Sel[32 * gi + k, k] = 1.0
    return W_rep, E2, Sel


def _build_nc(mm_dt, np_mm_dt):
    """Build the Bass module (same NEFF for all 8 cores)."""
    _patch_to_json()
    nc = bass.Bass("TRN2", enable_partition_id=False)
    x_in = nc.dram_tensor("x", (NS, F), F32, kind="ExternalInput")
    # c64: [xT | Tp] packed; c128: [W | E2 | Sel] packed (1 DMA each)
    c64_in = nc.dram_tensor("c64", (F, NS + KD), mm_dt, kind="ExternalInput")
    c128_in = nc.dram_tensor(
        "c128", (128, NCHUNK * 128 + NCHUNK * K + K), mm_dt, kind="ExternalInput"
    )
    out = nc.dram_tensor("out", (NS, F + K), F32, kind="ExternalOutput")

    with tile.TileContext(nc) as tc, ExitStack() as ctx:
        consts = ctx.enter_context(tc.tile_pool(name="consts", bufs=1))
        mt2_pool = ctx.enter_context(tc.tile_pool(name="mt2", bufs=4))
        pabs_pool = ctx.enter_context(tc.tile_pool(name="pabs", bufs=10))
        exp_pool = ctx.enter_context(tc.tile_pool(name="exps", bufs=2))
        misc_pool = ctx.enter_context(tc.tile_pool(name="misc", bufs=2))
        mm1_ps = ctx.enter_context(tc.tile_pool(name="mm1ps", bufs=7, space="PSUM"))
        a_ps = ctx.enter_context(tc.tile_pool(name="aps", bufs=1, space="PSUM"))

        # ---- constants / inputs to SBUF (matmul operands first) ----
        c64_sb = consts.tile([F, NS + KD], mm_dt)
        nc.sync.dma_start(out=c64_sb[:], in_=c64_in[:, :])
        c128_sb = consts.tile([128, NCHUNK * 128 + NCHUNK * K + K], mm_dt)
        nc.sync.dma_start(out=c128_sb[:], in_=c128_in[:, :])
        xT_sb = c64_sb[:, 0:NS]
        tp_sb = c64_sb[:, NS : NS + KD]
        w_sb = c128_sb[:, 0 : NCHUNK * 128]
        e2_sb = c128_sb[:, NCHUNK * 128 : NCHUNK * 128 + NCHUNK * K]
        sel_sb = c128_sb[:, NCHUNK * 128 + NCHUNK * K :]
        ident = consts.tile([128, 128], F32)

        # x passthrough: HBM -> HBM, fully off the critical path
        nc.sync.dma_start(out=out[:, 0:F], in_=x_in[:, :])

        # ---- M_T2[(d*32+j), n] = sum_f Tp[f, d*32+j] * xT[f, n] ----
        # Two MMs per double-wide PSUM slot; one wide copy-cast drains both.
        mt2_sb = []
        for q in range(4):
            ps = mm1_ps.tile([128, NS], F32, tag="mm1", name=f"mt2ps_{q}")
            nc.tensor.matmul(
                ps[:], lhsT=tp_sb[:, q * 128 : (q + 1) * 128], rhs=xT_sb[:],
                start=True, stop=True,
            )
            m = mt2_pool.tile([128, NS], mm_dt, tag="mt2", name=f"mt2_{q}")
            if q % 2 == 0:
                nc.scalar.copy(out=m[:], in_=ps[:])
            else:
                nc.vector.tensor_copy(out=m[:], in_=ps[:])
            mt2_sb.append(m)

        def mt2_slice(q, r):
            return mt2_sb[q][32 * r : 32 * r + 32, :]

        # ---- main loop: per d-group q (d = 4q+r), per chunk c ----
        # Software pipeline per chunk: MM1 quad (4-way row-packed, four
        # single-bank PSUM slots) -> four |.| ops (ACT/DVE alternating) ->
        # MM2 quad (4-way col-packed) one chunk behind.  feats accumulates
        # in SBUF so all 7 non-A PSUM banks go to the MM1 rotation.
        feats_sb = misc_pool.tile([K, NS], F32, tag="feats_sb")
        pend = []  # (q, c, [pabs tiles]) awaiting MM2 quad
        a_banks = {}
        mm2_done = {q: 0 for q in range(4)}

        def emit_mm2_quad():
            qq, cc, pps = pend.pop(0)
            for r in range(4):
                nc.tensor.matmul(
                    a_banks[qq][32 * r : 32 * r + 32, :],
                    lhsT=e2_sb[:, cc * K : (cc + 1) * K],
                    rhs=pps[r][:],
                    start=(cc == 0), stop=(cc == NCHUNK - 1),
                    tile_position=(0, 32 * r),
                )
            mm2_done[qq] += 1
            if mm2_done[qq] == NCHUNK:
                # A(q) complete: exp(-A), d-sum via matmul, accumulate feats
                ex = exp_pool.tile([128, NS], mm_dt, tag="exps")
                nc.scalar.activation(
                    out=ex[:], in_=a_banks[qq][:],
                    func=mybir.ActivationFunctionType.Exp, scale=-1.0,
                )
                fp = mm1_ps.tile([128, NS], F32, tag="mm1", name=f"fps_{qq}")
                nc.tensor.matmul(
                    fp[:K, :], lhsT=sel_sb[:], rhs=ex[:],
                    start=True, stop=True,
                )
                if qq == 0:
                    nc.vector.tensor_copy(out=feats_sb[:], in_=fp[:K, :])
                else:
                    nc.vector.tensor_tensor(
                        out=feats_sb[:], in0=feats_sb[:], in1=fp[:K, :],
                        op=mybir.AluOpType.add,
                    )

        for q in range(4):
            a_banks[q] = a_ps.tile([128, NS], F32, tag="abank", name=f"abank_{q}")
            for c in range(NCHUNK):
                pps = []
                for r in range(4):
                    p1 = mm1_ps.tile(
                        [128, NS], F32, tag="mm1", name=f"mm1_{q}_{c}_{r}"
                    )
                    nc.tensor.matmul(
                        p1[:],
                        lhsT=w_sb[32 * r : 32 * r + 32, c * 128 : (c + 1) * 128],
                        rhs=mt2_slice(q, r),
                        start=True, stop=True,
                        tile_position=(32 * r, 0),
                    )
                    pa = pabs_pool.tile(
                        [128, NS], mm_dt, tag="pabs", name=f"pabs_{q}_{c}_{r}"
                    )
                    if r % 2 == 0:
                        nc.scalar.activation(
                            out=pa[:], in_=p1[:],
                            func=mybir.ActivationFunctionType.Abs,
                        )
                    else:
                        with nc.allow_low_precision(reason="abs via 1-elem reduce"):
                            nc.vector.tensor_reduce(
                                out=pa[:],
                                in_=p1[:].rearrange("p (n o) -> p n o", o=1),
                                axis=mybir.AxisListType.X,
                                op=mybir.AluOpType.add,
                                apply_absolute_value=True,
                            )
                    pps.append(pa)
                pend.append((q, c, pps))
                if len(pend) > 1:
                    emit_mm2_quad()
        while pend:
            emit_mm2_quad()

        # ---- feats (K, NS) -> out[:, F:F+K] ----
        make_identity(nc, ident[:])
        fstage = misc_pool.tile([128, 4, K], F32, tag="fstage")
        for t in range(4):
            tp = mm1_ps.tile([128, NS], F32, tag="mm1", name=f"tp_{t}")
            nc.tensor.transpose(
                tp[:, :K], feats_sb[:, t * 128 : (t + 1) * 128], ident[:K, :K]
            )
            if t % 2 == 0:
                nc.vector.tensor_copy(out=fstage[:, t, :], in_=tp[:, :K])
            else:
                nc.scalar.copy(out=fstage[:, t, :], in_=tp[:, :K])
        nc.sync.dma_start(
            out=out[:, :].rearrange("(t p) f -> p t f", p=128)[:, :, F : F + K],
            in_=fstage[:],
        )
    return nc


_CACHED = {}


def _get_nc(use_bf16):
    key = ("bf16" if use_bf16 else "f32",)
    if key not in _CACHED:
        mm_dt = BF16 if use_bf16 else F32
        np_dt = NPBF16 if use_bf16 else np.float32
        _CACHED[key] = (_build_nc(mm_dt, np_dt), np_dt)
    return _CACHED[key]


def kernel(x, T, num_kernels, kernel_dim):
    assert int(num_kernels) == K and int(kernel_dim) == D
    x = np.asarray(x, dtype=np.float32)
    T = np.asarray(T, dtype=np.float32)
    B, S, f = x.shape
    assert (B, S, f) == (8, 512, 64) and T.shape == (F, KD)

    use_bf16 = os.environ.get("MBD_MM_DTYPE", "f32") == "bf16"
    nc, np_dt = _get_nc(use_bf16)

    # T_perm[f, d*32 + k] = T[f, k*16 + d]
    T_perm = T.reshape(F, K, D).transpose(0, 2, 1).reshape(F, KD)
    W_rep, E2, Sel = _host_constants()
    c128 = np.ascontiguousarray(
        np.concatenate([W_rep, E2, Sel], axis=1).astype(np_dt)
    )

    in_maps = []
    for c in range(NCORES):
        xc = np.ascontiguousarray(x[c])
        c64 = np.ascontiguousarray(
            np.concatenate([xc.T, T_perm], axis=1).astype(np_dt)
        )
        in_maps.append({"x": xc, "c64": c64, "c128": c128})

    trace = os.environ.get("MBD_TRACE", "0") == "1"
    res = run_bass_kernel_spmd(
        nc, in_maps, core_ids=list(range(NCORES)), trace=trace
    )
    kernel.last_results = res
    return np.stack([res.results[c]["out"] for c in range(NCORES)], axis=0)
